# revision 15
# baseline (speedup 1.0000x reference)
"""Trainium2 Bass kernel for gated inception-conv attention (8 cores, seq-parallel).

Shapes (hardcoded): q_data/k_data (1,8,1024,512) f32, bias (1,8,1024,1024) f32,
k_mask (1,8,1024) i32, Wq/Wk/Wv/Wg (512,512), bg (512), Wo (512,512), bo (512),
qcw/kcw/vcw (64,1,3), qcb/kcb/vcb (64).  Output (1,8,1024,512) f32.

v2: software-pipelined — projection chunk c runs concurrently with attention
for head-pair hp=c-1 (attention for hp only needs channel chunk hp).  k-mask
is folded into the host-precomputed exp(bias) (zeroed rows), so the exp has
no bias operand.  Scores kept transposed (L_k on partitions); softmax sum
rides the AV matmul (ones column on V).  All-bf16 compute, f32 PSUM.
"""

import os
import sys

sys.path.insert(0, "/opt/trn_rl_repo")

import numpy as np
import ml_dtypes

import concourse.bass as bass
import concourse.mybir as mybir
from concourse import bacc, tile
from concourse.bass_utils import run_bass_kernel_spmd

BF16 = ml_dtypes.bfloat16
F32 = mybir.dt.float32
BF16D = mybir.dt.bfloat16
FP = mybir.ActivationFunctionType
MULT = mybir.AluOpType.mult
ADD = mybir.AluOpType.add

H, D, L, C = 8, 64, 1024, 512
KD = VD = 512
NCORES = 8


def build():
    nc = bacc.Bacc(
        "TRN2",
        target_bir_lowering=False,
        debug=False,
        enable_asserts=False,
    )

    # ---- DRAM I/O (host pre-laid-out) ----
    qd = nc.dram_tensor("qd", [128, 4, L], BF16D, kind="ExternalInput").ap()
    kd = nc.dram_tensor("kd", [128, 4, L], BF16D, kind="ExternalInput").ap()
    # masked exp(bias): [hp, h01, p, kc*1024 + q] bf16 (rows for masked k zeroed)
    ebias = nc.dram_tensor("ebias", [4, 2, 128, 8 * 1024], BF16D, kind="ExternalInput").ap()
    wq = nc.dram_tensor("wq", [128, 4, KD], BF16D, kind="ExternalInput").ap()
    wk = nc.dram_tensor("wk", [128, 4, KD], BF16D, kind="ExternalInput").ap()
    wv = nc.dram_tensor("wv", [128, 4, VD], BF16D, kind="ExternalInput").ap()
    wg = nc.dram_tensor("wg", [128, 4, VD], BF16D, kind="ExternalInput").ap()
    wo = nc.dram_tensor("wo", [128, 4, C], BF16D, kind="ExternalInput").ap()
    convw = nc.dram_tensor("convw", [128, 12], F32, kind="ExternalInput").ap()
    bgbo = nc.dram_tensor("bgbo", [128, 8], F32, kind="ExternalInput").ap()
    selc = nc.dram_tensor("selc", [2, 128], F32, kind="ExternalInput").ap()
    identv = nc.dram_tensor("identv", [128, 128], BF16D, kind="ExternalInput").ap()
    out = nc.dram_tensor("out", [C, L], BF16D, kind="ExternalOutput").ap()

    with tile.TileContext(nc) as tc, nc.allow_low_precision(
        reason="bf16 compute; rel-err budget 2e-2"
    ):
        _body(tc, locals())
    nc.compile()
    return nc


def _body(tc, t):
    nc = tc.nc
    qd, kd, ebias = t["qd"], t["kd"], t["ebias"]
    wq, wk, wv, wg, wo = t["wq"], t["wk"], t["wv"], t["wg"], t["wo"]
    convw, bgbo, selc, identv, out = (
        t["convw"], t["bgbo"], t["selc"], t["identv"], t["out"],
    )

    with tc.tile_pool(name="const", bufs=1) as const, \
         tc.tile_pool(name="big", bufs=1) as big, \
         tc.tile_pool(name="ep", bufs=4) as ep, \
         tc.tile_pool(name="dpp", bufs=2) as dpp, \
         tc.tile_pool(name="ebp", bufs=3) as ebp, \
         tc.tile_pool(name="inw", bufs=1) as inw, \
         tc.tile_pool(name="pad", bufs=4) as padp, \
         tc.tile_pool(name="cvtmp", bufs=3) as cvp, \
         tc.tile_pool(name="vc", bufs=2) as vcp, \
         tc.tile_pool(name="stp", bufs=2) as stp, \
         tc.tile_pool(name="gfp", bufs=2) as gfp, \
         tc.tile_pool(name="qkps", bufs=2, space="PSUM") as qkp, \
         tc.tile_pool(name="avps", bufs=2, space="PSUM") as avp:

        # ---- inputs, in consumption order ----
        w_sb = {}
        for nm in ("q", "k", "v", "g"):
            w_sb[nm] = inw.tile([128, 4, 512], BF16D, name=f"w{nm}", tag=f"w{nm}")
        qdT = inw.tile([128, 4, L], BF16D, name="qdT", tag="qdT")
        kdT = inw.tile([128, 4, L], BF16D, name="kdT", tag="kdT")
        nc.sync.dma_start(w_sb["q"][:], wq)
        for ks in range(4):
            nc.sync.dma_start(qdT[:, ks, :], qd[:, ks, :])
        nc.sync.dma_start(w_sb["k"][:], wk)
        for ks in range(4):
            nc.sync.dma_start(kdT[:, ks, :], kd[:, ks, :])
        nc.sync.dma_start(w_sb["v"][:], wv)
        nc.sync.dma_start(w_sb["g"][:], wg)

        convw_sb = const.tile([128, 12], F32, name="convw", tag="convw")
        nc.sync.dma_start(convw_sb[:], convw)
        identv_sb = const.tile([128, 128], BF16D, name="identv", tag="identv")
        nc.sync.dma_start(identv_sb[:], identv)
        selc_sb = const.tile([2, 128], F32, name="selc", tag="selc")
        nc.sync.dma_start(selc_sb[:], selc)
        bgbo_sb = const.tile([128, 8], F32, name="bgbo", tag="bgbo")
        nc.sync.dma_start(bgbo_sb[:], bgbo)
        wo_sb = const.tile([128, 4, C], BF16D, name="wo", tag="wo")
        nc.sync.dma_start(wo_sb[:], wo)

        # ---- persistent tensors ----
        qc_t = [big.tile([128, L], BF16D, name=f"qc{c}", tag=f"qc{c}") for c in range(4)]
        kc_t = [big.tile([128, L], BF16D, name=f"kc{c}", tag=f"kc{c}") for c in range(4)]
        vnat = big.tile([128, H, 8, D + 1], BF16D, name="vnat", tag="vnat")
        nc.vector.memset(vnat[:, :, :, D : D + 1], 1.0)
        gT = big.tile([128, 4, L], BF16D, name="gT", tag="gT")
        oT_all = big.tile([128, 4, L], F32, name="oT", tag="oT")
        og = big.tile([128, 4, L], BF16D, name="og", tag="og")
        out_sb = big.tile([128, 4, L], BF16D, name="outsb", tag="outsb")

        rhsT = {"q": qdT, "k": kdT, "v": kdT, "g": qdT}
        cw = {"q": 0, "k": 4, "v": 8}
        pads_of = {}   # (c) -> dict nm -> pad tile
        vc_of = {}     # (c) -> vc tile

        def proj_mm(nm, c, dst, dst_off):
            """Project tensor nm chunk c; write bf16 to dst[:, dst_off+q2*512]."""
            for q2 in range(2):
                ps = qkp.tile([128, 512], F32, name=f"ps{nm}", tag="qk")
                for ks in range(4):
                    nc.tensor.matmul(
                        ps[:],
                        w_sb[nm][:, ks, c * 128 : (c + 1) * 128],
                        rhsT[nm][:, ks, q2 * 512 : (q2 + 1) * 512],
                        start=(ks == 0),
                        stop=(ks == 3),
                    )
                if nm == "g":
                    nc.scalar.activation(
                        gT[:, c, q2 * 512 : (q2 + 1) * 512], ps[:], FP.Sigmoid,
                        bias=bgbo_sb[:, c : c + 1],
                    )
                else:
                    nc.vector.tensor_copy(
                        dst[:, dst_off + q2 * 512 : dst_off + (q2 + 1) * 512], ps[:]
                    )

        def conv(nm, c, y):
            """Depthwise 3-tap conv from pads_of[c][nm] into y ([128, L] bf16)."""
            base = cw[nm]
            w0 = convw_sb[:, base : base + 1]
            w1 = convw_sb[:, base + 1 : base + 2]
            w2 = convw_sb[:, base + 2 : base + 3]
            bb = convw_sb[:, base + 3 : base + 4]
            x = pads_of[c][nm]
            nc.vector.tensor_scalar(y[:], x[:, 1 : L + 1], w1, bb, MULT, ADD)
            tm = cvp.tile([128, L], BF16D, name="cvtmp", tag="cvtmp")
            nc.vector.tensor_scalar_mul(tm[:], x[:, 0:L], w0)
            nc.vector.tensor_tensor(y[:], y[:], tm[:], ADD)
            tm2 = cvp.tile([128, L], BF16D, name="cvtmp2", tag="cvtmp")
            nc.vector.tensor_scalar_mul(tm2[:], x[:, 2 : L + 2], w2)
            nc.vector.tensor_tensor(y[:], y[:], tm2[:], ADD)

        def proj_slice(c, b):
            if b == 0:
                pads = {}
                for nm in ("q", "k", "v"):
                    pads[nm] = padp.tile(
                        [128, L + 2], BF16D, name=f"pad{nm}", tag=f"pad{nm}"
                    )
                    nc.vector.memset(pads[nm][:, 0:1], 0.0)
                    nc.vector.memset(pads[nm][:, L + 1 : L + 2], 0.0)
                pads_of[c] = pads
                proj_mm("q", c, pads["q"], 1)
            elif b == 1:
                proj_mm("k", c, pads_of[c]["k"], 1)
                conv("q", c, qc_t[c])
            elif b == 2:
                proj_mm("v", c, pads_of[c]["v"], 1)
                conv("k", c, kc_t[c])
            else:
                proj_mm("g", c, None, 0)
                vc = vcp.tile([128, L], BF16D, name="vc", tag="vc")
                vc_of[c] = vc
                conv("v", c, vc)
                # v -> natural layout: transpose 128x128 blocks (both heads)
                pst = qkp.tile([128, 8, 128], BF16D, name="pst", tag="qk")
                for bb_ in range(8):
                    nc.tensor.transpose(
                        pst[:, bb_, :],
                        vc[:, bb_ * 128 : (bb_ + 1) * 128],
                        identv_sb[:],
                    )
                for h01 in range(2):
                    nc.vector.tensor_copy(
                        vnat[:, 2 * c + h01, :, 0:D],
                        pst[:, :, h01 * 64 : (h01 + 1) * 64],
                    )

        av_of = {}  # hp -> [av_q0, av_q1]

        def attn_blk(hp, blk):
            if blk == 0:
                av_of[hp] = [
                    avp.tile([65, 2, 512], F32, name=f"av{q2}", tag="av")
                    for q2 in range(2)
                ]
            eT = [
                ep.tile([128, 2, L], BF16D, name=f"eT{h01}", tag=f"eT{h01}")
                for h01 in range(2)
            ]
            for kk in range(2):
                kc = blk * 2 + kk
                for h01 in range(2):
                    ph = h01 * 64
                    qk = qkp.tile([128, L], F32, name="qk", tag="qk")
                    for q2 in range(2):
                        nc.tensor.matmul(
                            qk[:, q2 * 512 : (q2 + 1) * 512],
                            kc_t[hp][ph : ph + 64, kc * 128 : (kc + 1) * 128],
                            qc_t[hp][ph : ph + 64, q2 * 512 : (q2 + 1) * 512],
                            start=True,
                            stop=True,
                        )
                    nc.scalar.activation(eT[h01][:, kk, :], qk[:], FP.Exp)
            for h01 in range(2):
                eb = ebp.tile([128, 2 * L], BF16D, name="ebst", tag="ebst")
                nc.sync.dma_start(
                    eb[:], ebias[hp, h01, :, blk * 2048 : (blk + 1) * 2048]
                )
                nc.vector.tensor_tensor(
                    eT[h01][:],
                    eT[h01][:],
                    eb.rearrange("p (kk q) -> p kk q", kk=2),
                    MULT,
                )
            for kk in range(2):
                kc = blk * 2 + kk
                for h01 in range(2):
                    for q2 in range(2):
                        nc.tensor.matmul(
                            av_of[hp][q2][:, h01, :],
                            vnat[:, 2 * hp + h01, kc, :],
                            eT[h01][:, kk, q2 * 512 : (q2 + 1) * 512],
                            start=(kc == 0),
                            stop=(kc == 7),
                        )

        def attn_av(hp):
            Dpt = dpp.tile([2, L], F32, name="Dp", tag="Dp")
            for q2 in range(2):
                st = stp.tile([65, 2, 512], F32, name="st", tag="st")
                nc.vector.tensor_copy(st[:], av_of[hp][q2][:])
                for h01 in range(2):
                    nc.sync.dma_start(
                        oT_all[
                            h01 * 64 : h01 * 64 + 64, hp,
                            q2 * 512 : (q2 + 1) * 512,
                        ],
                        st[0:64, h01, :],
                    )
                for h01 in range(2):
                    nc.sync.dma_start(
                        Dpt[h01 : h01 + 1, q2 * 512 : (q2 + 1) * 512],
                        st[64:65, h01, :],
                    )
            nc.vector.reciprocal_approx_fast(Dpt[:], Dpt[:])
            # normalize + gate this head-pair's output -> og (bf16)
            for q2 in range(2):
                rt = qkp.tile([128, 512], F32, name="rt", tag="qk")
                nc.tensor.matmul(
                    rt[:],
                    selc_sb[:],
                    Dpt[:, q2 * 512 : (q2 + 1) * 512],
                    start=True,
                    stop=True,
                )
                gf = gfp.tile([128, 512], BF16D, name="gf", tag="gf")
                nc.vector.tensor_tensor(
                    gf[:], gT[:, hp, q2 * 512 : (q2 + 1) * 512], rt[:], MULT
                )
                nc.vector.tensor_tensor(
                    og[:, hp, q2 * 512 : (q2 + 1) * 512],
                    oT_all[:, hp, q2 * 512 : (q2 + 1) * 512],
                    gf[:],
                    MULT,
                )

        # ================= pipelined main loop =================
        for c in range(5):
            for b in range(4):
                if c < 4:
                    proj_slice(c, b)
                if c >= 1:
                    attn_blk(c - 1, b)
            if c >= 1:
                attn_av(c - 1)

        # ================= output projection =================
        for mc in range(4):
            for q2 in range(2):
                pso = qkp.tile([128, 512], F32, name="ops", tag="qk")
                for ks in range(4):
                    nc.tensor.matmul(
                        pso[:],
                        wo_sb[:, ks, mc * 128 : (mc + 1) * 128],
                        og[:, ks, q2 * 512 : (q2 + 1) * 512],
                        start=(ks == 0),
                        stop=(ks == 3),
                    )
                nc.vector.tensor_scalar_add(
                    out_sb[:, mc, q2 * 512 : (q2 + 1) * 512], pso[:],
                    bgbo_sb[:, 4 + mc : 5 + mc],
                )
            nc.sync.dma_start(out[mc * 128 : (mc + 1) * 128, :], out_sb[:, mc, :])


# ---------------------------------------------------------------------------
# host side
# ---------------------------------------------------------------------------
_NC = None


def _get_nc():
    global _NC
    if _NC is None:
        _NC = build()
    return _NC


def _chunked(w):
    """(512, N) -> (128, 4, N) with row r at [r % 128, r // 128]."""
    n = np.asarray(w).shape[1]
    return np.ascontiguousarray(
        np.asarray(w, np.float32).reshape(4, 128, n).transpose(1, 0, 2)
    ).astype(BF16)


def _ctrans(x):
    """(L, C) -> (128, 4, L) bf16 with channel r at [r % 128, r // 128]."""
    xT = np.asarray(x, np.float32).T  # (C, L)
    return np.ascontiguousarray(
        xT.reshape(4, 128, L).transpose(1, 0, 2)
    ).astype(BF16)


def _prep_inmaps(q_data, k_data, bias, k_mask, Wq, Wk, Wv, Wg, bg, Wo, bo,
                 qcw, qcb, kcw, kcb, vcw, vcb):
    f32 = np.float32
    # exp(bias) -> [hp, h01, p, kc, q] f32: = exp(bias[0, 2hp+h01, q, kc*128+p])
    ebT = np.exp(np.asarray(bias[0], f32)).transpose(0, 2, 1)  # (h, k, q)
    # (h, k, q) -> (hp, h01, kc, p, q) -> (hp, h01, p, kc, q)
    eb_base = np.ascontiguousarray(
        ebT.reshape(4, 2, 8, 128, L).transpose(0, 1, 3, 2, 4)
    )  # (hp, h01, p, kc, q) f32

    wq_a, wk_a, wv_a, wg_a, wo_a = (_chunked(w) for w in (Wq, Wk, Wv, Wg, Wo))
    bgbo = np.zeros((128, 8), f32)
    bgbo[:, 0:4] = np.asarray(bg, f32).reshape(4, 128).T
    bgbo[:, 4:8] = np.asarray(bo, f32).reshape(4, 128).T

    selc = np.zeros((2, 128), f32)
    for m in range(128):
        selc[m // 64, m] = 1.0

    identv = np.eye(128, dtype=f32).astype(BF16)

    # conv taps per core: identity for seqs 0-3, real for 4-7; q scaled D^-0.5
    scale = 1.0 / np.sqrt(D)
    dd = np.arange(128) % 64

    def taps(w3, b1, use_real, s):
        cwc = np.zeros((128, 4), f32)
        if use_real:
            cwc[:, 0:3] = np.asarray(w3, f32)[dd, 0, :] * s
            cwc[:, 3] = np.asarray(b1, f32)[dd] * s
        else:
            cwc[:, 1] = s
        return cwc

    in_maps = []
    for s in range(NCORES):
        real = s >= 4
        cwm = np.concatenate(
            [
                taps(qcw, qcb, real, scale),
                taps(kcw, kcb, real, 1.0),
                taps(vcw, vcb, real, 1.0),
            ],
            axis=1,
        ).astype(f32)
        mk = np.asarray(k_mask[0, s], np.int32).reshape(8, 128).T  # (p, kc)
        ebm = (eb_base * mk.astype(f32)[None, None, :, :, None]).astype(BF16)
        in_maps.append(
            {
                "qd": _ctrans(q_data[0, s]),
                "kd": _ctrans(k_data[0, s]),
                "ebias": ebm.reshape(4, 2, 128, 8 * L),
                "wq": wq_a, "wk": wk_a, "wv": wv_a, "wg": wg_a, "wo": wo_a,
                "convw": cwm,
                "bgbo": bgbo,
                "selc": selc,
                "identv": identv,
            }
        )
    return in_maps


def run(in_maps, trace=False):
    nc = _get_nc()
    return run_bass_kernel_spmd(
        nc, in_maps, core_ids=list(range(NCORES)), trace=trace
    )


def kernel(**inputs):
    in_maps = _prep_inmaps(**inputs)
    res = run(in_maps)
    outp = np.empty((1, NCORES, L, C), np.float32)
    for s in range(NCORES):
        outp[0, s] = np.asarray(res.results[s]["out"], np.float32).T
    return outp


# revision 20
# speedup vs baseline: 1.0078x; 1.0078x over previous
"""Trainium2 Bass kernel for gated inception-conv attention (8 cores, seq-parallel).

Shapes (hardcoded): q_data/k_data (1,8,1024,512) f32, bias (1,8,1024,1024) f32,
k_mask (1,8,1024) i32, Wq/Wk/Wv/Wg (512,512), bg (512), Wo (512,512), bo (512),
qcw/kcw/vcw (64,1,3), qcb/kcb/vcb (64).  Output (1,8,1024,512) f32.

v2: software-pipelined — projection chunk c runs concurrently with attention
for head-pair hp=c-1 (attention for hp only needs channel chunk hp).  k-mask
is folded into the host-precomputed exp(bias) (zeroed rows), so the exp has
no bias operand.  Scores kept transposed (L_k on partitions); softmax sum
rides the AV matmul (ones column on V).  All-bf16 compute, f32 PSUM.
"""

import os
import sys

sys.path.insert(0, "/opt/trn_rl_repo")

import numpy as np
import ml_dtypes

import concourse.bass as bass
import concourse.mybir as mybir
from concourse import bacc, tile
from concourse.bass_utils import run_bass_kernel_spmd

BF16 = ml_dtypes.bfloat16
F32 = mybir.dt.float32
BF16D = mybir.dt.bfloat16
FP = mybir.ActivationFunctionType
MULT = mybir.AluOpType.mult
ADD = mybir.AluOpType.add

H, D, L, C = 8, 64, 1024, 512
KD = VD = 512
NCORES = 8


def build():
    nc = bacc.Bacc(
        "TRN2",
        target_bir_lowering=False,
        debug=False,
        enable_asserts=False,
    )

    # ---- DRAM I/O (host pre-laid-out) ----
    qd = nc.dram_tensor("qd", [128, 4, L], BF16D, kind="ExternalInput").ap()
    kd = nc.dram_tensor("kd", [128, 4, L], BF16D, kind="ExternalInput").ap()
    # masked exp(bias): [hp, h01, p, kc*1024 + q] bf16 (rows for masked k zeroed)
    ebias = nc.dram_tensor("ebias", [4, 2, 128, 8 * 1024], BF16D, kind="ExternalInput").ap()
    wq = nc.dram_tensor("wq", [128, 4, KD], BF16D, kind="ExternalInput").ap()
    wk = nc.dram_tensor("wk", [128, 4, KD], BF16D, kind="ExternalInput").ap()
    wv = nc.dram_tensor("wv", [128, 4, VD], BF16D, kind="ExternalInput").ap()
    wg = nc.dram_tensor("wg", [128, 4, VD], BF16D, kind="ExternalInput").ap()
    wo = nc.dram_tensor("wo", [128, 4, C], BF16D, kind="ExternalInput").ap()
    convw = nc.dram_tensor("convw", [128, 12], F32, kind="ExternalInput").ap()
    bgbo = nc.dram_tensor("bgbo", [128, 8], F32, kind="ExternalInput").ap()
    selc = nc.dram_tensor("selc", [2, 128], F32, kind="ExternalInput").ap()
    identv = nc.dram_tensor("identv", [128, 128], BF16D, kind="ExternalInput").ap()
    out = nc.dram_tensor("out", [C, L], BF16D, kind="ExternalOutput").ap()

    with tile.TileContext(nc) as tc, nc.allow_low_precision(
        reason="bf16 compute; rel-err budget 2e-2"
    ):
        _body(tc, locals())
    nc.compile()
    return nc


def _body(tc, t):
    nc = tc.nc
    qd, kd, ebias = t["qd"], t["kd"], t["ebias"]
    wq, wk, wv, wg, wo = t["wq"], t["wk"], t["wv"], t["wg"], t["wo"]
    convw, bgbo, selc, identv, out = (
        t["convw"], t["bgbo"], t["selc"], t["identv"], t["out"],
    )

    with tc.tile_pool(name="const", bufs=1) as const, \
         tc.tile_pool(name="big", bufs=1) as big, \
         tc.tile_pool(name="ep", bufs=4) as ep, \
         tc.tile_pool(name="dpp", bufs=2) as dpp, \
         tc.tile_pool(name="ebp", bufs=3) as ebp, \
         tc.tile_pool(name="inw", bufs=1) as inw, \
         tc.tile_pool(name="pad", bufs=4) as padp, \
         tc.tile_pool(name="cvtmp", bufs=3) as cvp, \
         tc.tile_pool(name="vc", bufs=2) as vcp, \
         tc.tile_pool(name="stp", bufs=2) as stp, \
         tc.tile_pool(name="gfp", bufs=2) as gfp, \
         tc.tile_pool(name="qkps", bufs=2, space="PSUM") as qkp, \
         tc.tile_pool(name="avps", bufs=2, space="PSUM") as avp:

        # ---- inputs, in consumption order ----
        w_sb = {}
        for nm in ("q", "k", "v", "g"):
            w_sb[nm] = inw.tile([128, 4, 512], BF16D, name=f"w{nm}", tag=f"w{nm}")
        qdT = inw.tile([128, 4, L], BF16D, name="qdT", tag="qdT")
        kdT = inw.tile([128, 4, L], BF16D, name="kdT", tag="kdT")
        nc.sync.dma_start(w_sb["q"][:], wq)
        for ks in range(4):
            nc.sync.dma_start(qdT[:, ks, :], qd[:, ks, :])
        nc.sync.dma_start(w_sb["g"][:], wg)
        nc.sync.dma_start(w_sb["k"][:], wk)
        for ks in range(4):
            nc.sync.dma_start(kdT[:, ks, :], kd[:, ks, :])
        nc.sync.dma_start(w_sb["v"][:], wv)

        convw_sb = const.tile([128, 12], F32, name="convw", tag="convw")
        nc.sync.dma_start(convw_sb[:], convw)
        identv_sb = const.tile([128, 128], BF16D, name="identv", tag="identv")
        nc.sync.dma_start(identv_sb[:], identv)
        selc_sb = const.tile([2, 128], F32, name="selc", tag="selc")
        nc.sync.dma_start(selc_sb[:], selc)
        bgbo_sb = const.tile([128, 8], F32, name="bgbo", tag="bgbo")
        nc.sync.dma_start(bgbo_sb[:], bgbo)
        wo_sb = const.tile([128, 4, C], BF16D, name="wo", tag="wo")
        nc.sync.dma_start(wo_sb[:], wo)

        # ---- persistent tensors ----
        qc_t = [big.tile([128, L], BF16D, name=f"qc{c}", tag=f"qc{c}") for c in range(4)]
        kc_t = [big.tile([128, L], BF16D, name=f"kc{c}", tag=f"kc{c}") for c in range(4)]
        vnat = big.tile([128, H, 8, D + 1], BF16D, name="vnat", tag="vnat")
        nc.vector.memset(vnat[:, :, :, D : D + 1], 1.0)
        gT = big.tile([128, 4, L], BF16D, name="gT", tag="gT")
        oT_all = big.tile([128, 4, L], F32, name="oT", tag="oT")
        og = big.tile([128, 4, L], BF16D, name="og", tag="og")
        out_sb = big.tile([128, 4, L], BF16D, name="outsb", tag="outsb")

        rhsT = {"q": qdT, "k": kdT, "v": kdT, "g": qdT}
        cw = {"q": 0, "k": 4, "v": 8}
        pads_of = {}   # (c) -> dict nm -> pad tile
        vc_of = {}     # (c) -> vc tile

        def proj_mm(nm, c, dst, dst_off):
            """Project tensor nm chunk c; write bf16 to dst[:, dst_off+q2*512]."""
            for q2 in range(2):
                ps = qkp.tile([128, 512], F32, name=f"ps{nm}", tag="qk")
                for ks in range(4):
                    nc.tensor.matmul(
                        ps[:],
                        w_sb[nm][:, ks, c * 128 : (c + 1) * 128],
                        rhsT[nm][:, ks, q2 * 512 : (q2 + 1) * 512],
                        start=(ks == 0),
                        stop=(ks == 3),
                    )
                if nm == "g":
                    nc.scalar.activation(
                        gT[:, c, q2 * 512 : (q2 + 1) * 512], ps[:], FP.Sigmoid,
                        bias=bgbo_sb[:, c : c + 1],
                    )
                else:
                    nc.vector.tensor_copy(
                        dst[:, dst_off + q2 * 512 : dst_off + (q2 + 1) * 512], ps[:]
                    )

        def conv(nm, c, y):
            """Depthwise 3-tap conv from pads_of[c][nm] into y ([128, L] bf16)."""
            base = cw[nm]
            w0 = convw_sb[:, base : base + 1]
            w1 = convw_sb[:, base + 1 : base + 2]
            w2 = convw_sb[:, base + 2 : base + 3]
            bb = convw_sb[:, base + 3 : base + 4]
            x = pads_of[c][nm]
            nc.vector.tensor_scalar(y[:], x[:, 1 : L + 1], w1, bb, MULT, ADD)
            tm = cvp.tile([128, L], BF16D, name="cvtmp", tag="cvtmp")
            nc.vector.tensor_scalar_mul(tm[:], x[:, 0:L], w0)
            nc.vector.tensor_tensor(y[:], y[:], tm[:], ADD)
            tm2 = cvp.tile([128, L], BF16D, name="cvtmp2", tag="cvtmp")
            nc.vector.tensor_scalar_mul(tm2[:], x[:, 2 : L + 2], w2)
            nc.vector.tensor_tensor(y[:], y[:], tm2[:], ADD)

        def proj_slice(c, b):
            if b == 0:
                pads = {}
                for nm in ("q", "k", "v"):
                    pads[nm] = padp.tile(
                        [128, L + 2], BF16D, name=f"pad{nm}", tag=f"pad{nm}"
                    )
                    nc.vector.memset(pads[nm][:, 0:1], 0.0)
                    nc.vector.memset(pads[nm][:, L + 1 : L + 2], 0.0)
                pads_of[c] = pads
                proj_mm("q", c, pads["q"], 1)
            elif b == 1:
                proj_mm("k", c, pads_of[c]["k"], 1)
                conv("q", c, qc_t[c])
            elif b == 2:
                proj_mm("v", c, pads_of[c]["v"], 1)
                conv("k", c, kc_t[c])
            else:
                vc = vcp.tile([128, L], BF16D, name="vc", tag="vc")
                vc_of[c] = vc
                conv("v", c, vc)
                # v -> natural layout: transpose 128x128 blocks (both heads)
                pst = qkp.tile([128, 8, 128], BF16D, name="pst", tag="qk")
                for bb_ in range(8):
                    nc.tensor.transpose(
                        pst[:, bb_, :],
                        vc[:, bb_ * 128 : (bb_ + 1) * 128],
                        identv_sb[:],
                    )
                for h01 in range(2):
                    nc.vector.tensor_copy(
                        vnat[:, 2 * c + h01, :, 0:D],
                        pst[:, :, h01 * 64 : (h01 + 1) * 64],
                    )

        av_of = {}  # hp -> [av_q0, av_q1]
        eT_of = {}  # (hp, blk) -> [eT_h0, eT_h1]

        def attn_qk(hp, blk):
            if blk == 0:
                av_of[hp] = [
                    avp.tile([65, 2, 512], F32, name=f"av{q2}", tag="av")
                    for q2 in range(2)
                ]
            eT = [
                ep.tile([128, 2, L], BF16D, name=f"eT{h01}", tag=f"eT{h01}")
                for h01 in range(2)
            ]
            eT_of[(hp, blk)] = eT
            for kk in range(2):
                kc = blk * 2 + kk
                for h01 in range(2):
                    ph = h01 * 64
                    qk = qkp.tile([128, L], F32, name="qk", tag="qk")
                    for q2 in range(2):
                        nc.tensor.matmul(
                            qk[:, q2 * 512 : (q2 + 1) * 512],
                            kc_t[hp][ph : ph + 64, kc * 128 : (kc + 1) * 128],
                            qc_t[hp][ph : ph + 64, q2 * 512 : (q2 + 1) * 512],
                            start=True,
                            stop=True,
                        )
                    nc.scalar.activation(eT[h01][:, kk, :], qk[:], FP.Exp)

        def attn_av(hp, blk):
            eT = eT_of.pop((hp, blk))
            for h01 in range(2):
                eb = ebp.tile([128, 2 * L], BF16D, name="ebst", tag="ebst")
                nc.sync.dma_start(
                    eb[:], ebias[hp, h01, :, blk * 2048 : (blk + 1) * 2048]
                )
                nc.vector.tensor_tensor(
                    eT[h01][:],
                    eT[h01][:],
                    eb.rearrange("p (kk q) -> p kk q", kk=2),
                    MULT,
                )
            for kk in range(2):
                kc = blk * 2 + kk
                for h01 in range(2):
                    for q2 in range(2):
                        nc.tensor.matmul(
                            av_of[hp][q2][:, h01, :],
                            vnat[:, 2 * hp + h01, kc, :],
                            eT[h01][:, kk, q2 * 512 : (q2 + 1) * 512],
                            start=(kc == 0),
                            stop=(kc == 7),
                        )

        def attn_fin(hp):
            Dpt = dpp.tile([2, L], F32, name="Dp", tag="Dp")
            for q2 in range(2):
                st = stp.tile([65, 2, 512], F32, name="st", tag="st")
                nc.vector.tensor_copy(st[:], av_of[hp][q2][:])
                for h01 in range(2):
                    nc.sync.dma_start(
                        oT_all[
                            h01 * 64 : h01 * 64 + 64, hp,
                            q2 * 512 : (q2 + 1) * 512,
                        ],
                        st[0:64, h01, :],
                    )
                for h01 in range(2):
                    nc.sync.dma_start(
                        Dpt[h01 : h01 + 1, q2 * 512 : (q2 + 1) * 512],
                        st[64:65, h01, :],
                    )
            nc.vector.reciprocal_approx_fast(Dpt[:], Dpt[:])
            # normalize + gate this head-pair's output -> og (bf16)
            for q2 in range(2):
                rt = qkp.tile([128, 512], F32, name="rt", tag="qk")
                nc.tensor.matmul(
                    rt[:],
                    selc_sb[:],
                    Dpt[:, q2 * 512 : (q2 + 1) * 512],
                    start=True,
                    stop=True,
                )
                gf = gfp.tile([128, 512], BF16D, name="gf", tag="gf")
                nc.vector.tensor_tensor(
                    gf[:], gT[:, hp, q2 * 512 : (q2 + 1) * 512], rt[:], MULT
                )
                nc.vector.tensor_tensor(
                    og[:, hp, q2 * 512 : (q2 + 1) * 512],
                    oT_all[:, hp, q2 * 512 : (q2 + 1) * 512],
                    gf[:],
                    MULT,
                )

        # ===== prologue: all gate projections (one sigmoid table epoch) =====
        for c in range(4):
            proj_mm("g", c, None, 0)

        # ================= pipelined main loop =================
        for c in range(5):
            for b in range(4):
                if c >= 1:
                    attn_qk(c - 1, b)
                if c < 4:
                    proj_slice(c, b)
                if c >= 1:
                    attn_av(c - 1, b)
            if c >= 1:
                attn_fin(c - 1)

        # ================= output projection =================
        for mc in range(4):
            for q2 in range(2):
                pso = qkp.tile([128, 512], F32, name="ops", tag="qk")
                for ks in range(4):
                    nc.tensor.matmul(
                        pso[:],
                        wo_sb[:, ks, mc * 128 : (mc + 1) * 128],
                        og[:, ks, q2 * 512 : (q2 + 1) * 512],
                        start=(ks == 0),
                        stop=(ks == 3),
                    )
                nc.vector.tensor_scalar_add(
                    out_sb[:, mc, q2 * 512 : (q2 + 1) * 512], pso[:],
                    bgbo_sb[:, 4 + mc : 5 + mc],
                )
            nc.sync.dma_start(out[mc * 128 : (mc + 1) * 128, :], out_sb[:, mc, :])


# ---------------------------------------------------------------------------
# host side
# ---------------------------------------------------------------------------
_NC = None


def _get_nc():
    global _NC
    if _NC is None:
        _NC = build()
    return _NC


def _chunked(w):
    """(512, N) -> (128, 4, N) with row r at [r % 128, r // 128]."""
    n = np.asarray(w).shape[1]
    return np.ascontiguousarray(
        np.asarray(w, np.float32).reshape(4, 128, n).transpose(1, 0, 2)
    ).astype(BF16)


def _ctrans(x):
    """(L, C) -> (128, 4, L) bf16 with channel r at [r % 128, r // 128]."""
    xT = np.asarray(x, np.float32).T  # (C, L)
    return np.ascontiguousarray(
        xT.reshape(4, 128, L).transpose(1, 0, 2)
    ).astype(BF16)


def _prep_inmaps(q_data, k_data, bias, k_mask, Wq, Wk, Wv, Wg, bg, Wo, bo,
                 qcw, qcb, kcw, kcb, vcw, vcb):
    f32 = np.float32
    # exp(bias) -> [hp, h01, p, kc, q] f32: = exp(bias[0, 2hp+h01, q, kc*128+p])
    ebT = np.exp(np.asarray(bias[0], f32)).transpose(0, 2, 1)  # (h, k, q)
    # (h, k, q) -> (hp, h01, kc, p, q) -> (hp, h01, p, kc, q)
    eb_base = np.ascontiguousarray(
        ebT.reshape(4, 2, 8, 128, L).transpose(0, 1, 3, 2, 4)
    )  # (hp, h01, p, kc, q) f32

    wq_a, wk_a, wv_a, wg_a, wo_a = (_chunked(w) for w in (Wq, Wk, Wv, Wg, Wo))
    bgbo = np.zeros((128, 8), f32)
    bgbo[:, 0:4] = np.asarray(bg, f32).reshape(4, 128).T
    bgbo[:, 4:8] = np.asarray(bo, f32).reshape(4, 128).T

    selc = np.zeros((2, 128), f32)
    for m in range(128):
        selc[m // 64, m] = 1.0

    identv = np.eye(128, dtype=f32).astype(BF16)

    # conv taps per core: identity for seqs 0-3, real for 4-7; q scaled D^-0.5
    scale = 1.0 / np.sqrt(D)
    dd = np.arange(128) % 64

    def taps(w3, b1, use_real, s):
        cwc = np.zeros((128, 4), f32)
        if use_real:
            cwc[:, 0:3] = np.asarray(w3, f32)[dd, 0, :] * s
            cwc[:, 3] = np.asarray(b1, f32)[dd] * s
        else:
            cwc[:, 1] = s
        return cwc

    in_maps = []
    for s in range(NCORES):
        real = s >= 4
        cwm = np.concatenate(
            [
                taps(qcw, qcb, real, scale),
                taps(kcw, kcb, real, 1.0),
                taps(vcw, vcb, real, 1.0),
            ],
            axis=1,
        ).astype(f32)
        mk = np.asarray(k_mask[0, s], np.int32).reshape(8, 128).T  # (p, kc)
        ebm = (eb_base * mk.astype(f32)[None, None, :, :, None]).astype(BF16)
        in_maps.append(
            {
                "qd": _ctrans(q_data[0, s]),
                "kd": _ctrans(k_data[0, s]),
                "ebias": ebm.reshape(4, 2, 128, 8 * L),
                "wq": wq_a, "wk": wk_a, "wv": wv_a, "wg": wg_a, "wo": wo_a,
                "convw": cwm,
                "bgbo": bgbo,
                "selc": selc,
                "identv": identv,
            }
        )
    return in_maps


def run(in_maps, trace=False):
    nc = _get_nc()
    return run_bass_kernel_spmd(
        nc, in_maps, core_ids=list(range(NCORES)), trace=trace
    )


def kernel(**inputs):
    in_maps = _prep_inmaps(**inputs)
    res = run(in_maps)
    outp = np.empty((1, NCORES, L, C), np.float32)
    for s in range(NCORES):
        outp[0, s] = np.asarray(res.results[s]["out"], np.float32).T
    return outp


# revision 28
# speedup vs baseline: 1.0311x; 1.0231x over previous
"""Trainium2 Bass kernel for gated inception-conv attention (8 cores, seq-parallel).

Shapes (hardcoded): q_data/k_data (1,8,1024,512) f32, bias (1,8,1024,1024) f32,
k_mask (1,8,1024) i32, Wq/Wk/Wv/Wg (512,512), bg (512), Wo (512,512), bo (512),
qcw/kcw/vcw (64,1,3), qcb/kcb/vcb (64).  Output (1,8,1024,512) f32.

v2: software-pipelined — projection chunk c runs concurrently with attention
for head-pair hp=c-1 (attention for hp only needs channel chunk hp).  k-mask
is folded into the host-precomputed exp(bias) (zeroed rows), so the exp has
no bias operand.  Scores kept transposed (L_k on partitions); softmax sum
rides the AV matmul (ones column on V).  All-bf16 compute, f32 PSUM.
"""

import os
import sys

sys.path.insert(0, "/opt/trn_rl_repo")

import numpy as np
import ml_dtypes

import concourse.bass as bass
import concourse.mybir as mybir
from concourse import bacc, tile
from concourse.bass_utils import run_bass_kernel_spmd

BF16 = ml_dtypes.bfloat16
F32 = mybir.dt.float32
BF16D = mybir.dt.bfloat16
FP = mybir.ActivationFunctionType
MULT = mybir.AluOpType.mult
ADD = mybir.AluOpType.add

H, D, L, C = 8, 64, 1024, 512
KD = VD = 512
NCORES = 8


def build():
    nc = bacc.Bacc(
        "TRN2",
        target_bir_lowering=False,
        debug=False,
        enable_asserts=False,
    )

    # ---- DRAM I/O (host pre-laid-out) ----
    qd = nc.dram_tensor("qd", [128, 4, L], BF16D, kind="ExternalInput").ap()
    kd = nc.dram_tensor("kd", [128, 4, L], BF16D, kind="ExternalInput").ap()
    # masked exp(bias): [hp, h01, p, kc*1024 + q] bf16 (rows for masked k zeroed)
    ebias = nc.dram_tensor("ebias", [4, 2, 128, 8 * 1024], BF16D, kind="ExternalInput").ap()
    wq = nc.dram_tensor("wq", [128, 4, KD], BF16D, kind="ExternalInput").ap()
    wk = nc.dram_tensor("wk", [128, 4, KD], BF16D, kind="ExternalInput").ap()
    wv = nc.dram_tensor("wv", [128, 4, VD], BF16D, kind="ExternalInput").ap()
    wg = nc.dram_tensor("wg", [128, 4, VD], BF16D, kind="ExternalInput").ap()
    wo = nc.dram_tensor("wo", [128, 4, C], BF16D, kind="ExternalInput").ap()
    convw = nc.dram_tensor("convw", [128, 12], F32, kind="ExternalInput").ap()
    bgbo = nc.dram_tensor("bgbo", [128, 8], F32, kind="ExternalInput").ap()
    identv = nc.dram_tensor("identv", [128, 128], BF16D, kind="ExternalInput").ap()
    selc = nc.dram_tensor("selc", [2, 128], F32, kind="ExternalInput").ap()
    out = nc.dram_tensor("out", [C, L], BF16D, kind="ExternalOutput").ap()

    with tile.TileContext(nc) as tc, nc.allow_low_precision(
        reason="bf16 compute; rel-err budget 2e-2"
    ):
        _body(tc, locals())
    nc.compile()
    return nc


def _body(tc, t):
    nc = tc.nc
    qd, kd, ebias = t["qd"], t["kd"], t["ebias"]
    wq, wk, wv, wg, wo = t["wq"], t["wk"], t["wv"], t["wg"], t["wo"]
    convw, bgbo, identv, selc, out = (
        t["convw"], t["bgbo"], t["identv"], t["selc"], t["out"],
    )

    with tc.tile_pool(name="const", bufs=1) as const, \
         tc.tile_pool(name="big", bufs=1) as big, \
         tc.tile_pool(name="ep", bufs=4) as ep, \
         tc.tile_pool(name="dpp", bufs=4) as dpp, \
         tc.tile_pool(name="ebp", bufs=3) as ebp, \
         tc.tile_pool(name="inw", bufs=1) as inw, \
         tc.tile_pool(name="pad", bufs=4) as padp, \
         tc.tile_pool(name="cvtmp", bufs=3) as cvp, \
         tc.tile_pool(name="vc", bufs=2) as vcp, \
         tc.tile_pool(name="stp", bufs=2) as stp, \
         tc.tile_pool(name="gfp", bufs=2) as gfp, \
         tc.tile_pool(name="qkps", bufs=2, space="PSUM") as qkp, \
         tc.tile_pool(name="avps", bufs=2, space="PSUM") as avp:

        # ---- inputs, in consumption order ----
        w_sb = {}
        for nm in ("q", "k", "v", "g"):
            w_sb[nm] = inw.tile([128, 4, 512], BF16D, name=f"w{nm}", tag=f"w{nm}")
        qdT = inw.tile([128, 4, L], BF16D, name="qdT", tag="qdT")
        kdT = inw.tile([128, 4, L], BF16D, name="kdT", tag="kdT")
        nc.sync.dma_start(w_sb["q"][:], wq)
        for ks in range(4):
            nc.sync.dma_start(qdT[:, ks, :], qd[:, ks, :])
        nc.sync.dma_start(w_sb["g"][:], wg)
        nc.sync.dma_start(w_sb["k"][:], wk)
        for ks in range(4):
            nc.sync.dma_start(kdT[:, ks, :], kd[:, ks, :])
        nc.sync.dma_start(w_sb["v"][:], wv)

        convw_sb = const.tile([128, 12], F32, name="convw", tag="convw")
        nc.sync.dma_start(convw_sb[:], convw)
        identv_sb = const.tile([128, 128], BF16D, name="identv", tag="identv")
        nc.sync.dma_start(identv_sb[:], identv)
        selc_sb = const.tile([2, 128], F32, name="selc", tag="selc")
        nc.sync.dma_start(selc_sb[:], selc)
        bgbo_sb = const.tile([128, 8], F32, name="bgbo", tag="bgbo")
        nc.sync.dma_start(bgbo_sb[:], bgbo)
        wo_sb = const.tile([128, 4, C], BF16D, name="wo", tag="wo")
        nc.sync.dma_start(wo_sb[:], wo)

        # ---- persistent tensors ----
        qc_t = [big.tile([128, L], BF16D, name=f"qc{c}", tag=f"qc{c}") for c in range(4)]
        kc_t = [big.tile([128, L], BF16D, name=f"kc{c}", tag=f"kc{c}") for c in range(4)]
        vnat = big.tile([128, H, 8, D + 1], BF16D, name="vnat", tag="vnat")
        nc.vector.memset(vnat[:, :, :, D : D + 1], 1.0)
        gT = big.tile([128, 4, L], BF16D, name="gT", tag="gT")
        oT_all = big.tile([128, 4, L], F32, name="oT", tag="oT")
        og = big.tile([128, 4, L], BF16D, name="og", tag="og")
        out_sb = big.tile([128, 4, L], BF16D, name="outsb", tag="outsb")

        rhsT = {"q": qdT, "k": kdT, "v": kdT, "g": qdT}
        cw = {"q": 0, "k": 4, "v": 8}
        pads_of = {}   # (c) -> dict nm -> pad tile
        vc_of = {}     # (c) -> vc tile

        def proj_mm(nm, c, dst, dst_off):
            """Project tensor nm chunk c; write bf16 to dst[:, dst_off+q2*512]."""
            for q2 in range(2):
                ps = qkp.tile([128, 512], F32, name=f"ps{nm}", tag="qk")
                for ks in range(4):
                    nc.tensor.matmul(
                        ps[:],
                        w_sb[nm][:, ks, c * 128 : (c + 1) * 128],
                        rhsT[nm][:, ks, q2 * 512 : (q2 + 1) * 512],
                        start=(ks == 0),
                        stop=(ks == 3),
                    )
                if nm == "g":
                    nc.scalar.activation(
                        gT[:, c, q2 * 512 : (q2 + 1) * 512], ps[:], FP.Sigmoid,
                        bias=bgbo_sb[:, c : c + 1],
                    )
                else:
                    nc.vector.tensor_copy(
                        dst[:, dst_off + q2 * 512 : dst_off + (q2 + 1) * 512], ps[:]
                    )

        def conv(nm, c, y):
            """Depthwise 3-tap conv from pads_of[c][nm] into y ([128, L] bf16)."""
            base = cw[nm]
            w0 = convw_sb[:, base : base + 1]
            w1 = convw_sb[:, base + 1 : base + 2]
            w2 = convw_sb[:, base + 2 : base + 3]
            bb = convw_sb[:, base + 3 : base + 4]
            x = pads_of[c][nm]
            nc.vector.tensor_scalar(y[:], x[:, 1 : L + 1], w1, bb, MULT, ADD)
            tm = cvp.tile([128, L], BF16D, name="cvtmp", tag="cvtmp")
            nc.vector.tensor_scalar_mul(tm[:], x[:, 0:L], w0)
            nc.vector.tensor_tensor(y[:], y[:], tm[:], ADD)
            tm2 = cvp.tile([128, L], BF16D, name="cvtmp2", tag="cvtmp")
            nc.vector.tensor_scalar_mul(tm2[:], x[:, 2 : L + 2], w2)
            nc.vector.tensor_tensor(y[:], y[:], tm2[:], ADD)

        def proj_slice(c, b):
            if b == 0:
                pads = {}
                for nm in ("q", "k", "v"):
                    pads[nm] = padp.tile(
                        [128, L + 2], BF16D, name=f"pad{nm}", tag=f"pad{nm}"
                    )
                    nc.vector.memset(pads[nm][:, 0:1], 0.0)
                    nc.vector.memset(pads[nm][:, L + 1 : L + 2], 0.0)
                pads_of[c] = pads
                proj_mm("q", c, pads["q"], 1)
            elif b == 1:
                proj_mm("k", c, pads_of[c]["k"], 1)
                conv("q", c, qc_t[c])
            elif b == 2:
                proj_mm("v", c, pads_of[c]["v"], 1)
                conv("k", c, kc_t[c])
            else:
                vc = vcp.tile([128, L], BF16D, name="vc", tag="vc")
                vc_of[c] = vc
                conv("v", c, vc)
                # v -> natural layout: transpose 128x128 blocks (both heads)
                pst = qkp.tile([128, 8, 128], BF16D, name="pst", tag="qk")
                for bb_ in range(8):
                    nc.tensor.transpose(
                        pst[:, bb_, :],
                        vc[:, bb_ * 128 : (bb_ + 1) * 128],
                        identv_sb[:],
                    )
                for h01 in range(2):
                    nc.vector.tensor_copy(
                        vnat[:, 2 * c + h01, :, 0:D],
                        pst[:, :, h01 * 64 : (h01 + 1) * 64],
                    )

        av_of = {}  # hp -> [av_q0, av_q1]
        Dpt_of = {}  # hp -> Dpt recip tile
        eT_of = {}  # (hp, blk) -> [eT_h0, eT_h1]

        def attn_qk(hp, blk):
            if blk == 0:
                av_of[hp] = [
                    avp.tile([65, 2, 512], F32, name=f"av{q2}", tag="av")
                    for q2 in range(2)
                ]
            eT = [
                ep.tile([128, 2, L], BF16D, name=f"eT{h01}", tag=f"eT{h01}")
                for h01 in range(2)
            ]
            eT_of[(hp, blk)] = eT
            for kk in range(2):
                kc = blk * 2 + kk
                for h01 in range(2):
                    ph = h01 * 64
                    qk = qkp.tile([128, L], F32, name="qk", tag="qk")
                    for q2 in range(2):
                        nc.tensor.matmul(
                            qk[:, q2 * 512 : (q2 + 1) * 512],
                            kc_t[hp][ph : ph + 64, kc * 128 : (kc + 1) * 128],
                            qc_t[hp][ph : ph + 64, q2 * 512 : (q2 + 1) * 512],
                            start=True,
                            stop=True,
                        )
                    nc.scalar.activation(eT[h01][:, kk, :], qk[:], FP.Exp)

        def attn_av(hp, blk):
            eT = eT_of.pop((hp, blk))
            for h01 in range(2):
                eb = ebp.tile([128, 2 * L], BF16D, name="ebst", tag="ebst")
                nc.sync.dma_start(
                    eb[:], ebias[hp, h01, :, blk * 2048 : (blk + 1) * 2048]
                )
                nc.vector.tensor_tensor(
                    eT[h01][:],
                    eT[h01][:],
                    eb.rearrange("p (kk q) -> p kk q", kk=2),
                    MULT,
                )
            for kk in range(2):
                kc = blk * 2 + kk
                for h01 in range(2):
                    for q2 in range(2):
                        nc.tensor.matmul(
                            av_of[hp][q2][:, h01, :],
                            vnat[:, 2 * hp + h01, kc, :],
                            eT[h01][:, kk, q2 * 512 : (q2 + 1) * 512],
                            start=(kc == 0),
                            stop=(kc == 7),
                        )

        def attn_fin(hp):
            Dpt = dpp.tile([2, L], F32, name="Dp", tag="Dp")
            for q2 in range(2):
                st = stp.tile([65, 2, 512], F32, name="st", tag="st")
                nc.vector.tensor_copy(st[:], av_of[hp][q2][:])
                for h01 in range(2):
                    nc.sync.dma_start(
                        oT_all[
                            h01 * 64 : h01 * 64 + 64, hp,
                            q2 * 512 : (q2 + 1) * 512,
                        ],
                        st[0:64, h01, :],
                    )
                for h01 in range(2):
                    nc.sync.dma_start(
                        Dpt[h01 : h01 + 1, q2 * 512 : (q2 + 1) * 512],
                        st[64:65, h01, :],
                    )
            nc.vector.reciprocal_approx_fast(Dpt[:], Dpt[:])
            Dpt_of[hp] = Dpt

        # ===== prologue: all gate projections (one sigmoid table epoch) =====
        for c in range(4):
            proj_mm("g", c, None, 0)

        # ================= pipelined main loop =================
        for c in range(5):
            for b in range(4):
                if c >= 1:
                    attn_qk(c - 1, b)
                if c < 4:
                    proj_slice(c, b)
                if c >= 1:
                    attn_av(c - 1, b)
            if c >= 1:
                attn_fin(c - 1)

        # ======== tail: normalize+gate all head-pairs, then out-proj ========
        for hp in range(4):
            for q2 in range(2):
                rt = qkp.tile([128, 512], F32, name="rt", tag="qk")
                nc.tensor.matmul(
                    rt[:],
                    selc_sb[:],
                    Dpt_of[hp][:, q2 * 512 : (q2 + 1) * 512],
                    start=True,
                    stop=True,
                )
                gf = gfp.tile([128, 512], BF16D, name="gf", tag="gf")
                nc.vector.tensor_tensor(
                    gf[:], gT[:, hp, q2 * 512 : (q2 + 1) * 512], rt[:], MULT
                )
                nc.vector.tensor_tensor(
                    og[:, hp, q2 * 512 : (q2 + 1) * 512],
                    oT_all[:, hp, q2 * 512 : (q2 + 1) * 512],
                    gf[:],
                    MULT,
                )

        # ================= output projection =================
        for mc in range(4):
            for q2 in range(2):
                pso = qkp.tile([128, 512], F32, name="ops", tag="qk")
                for ks in range(4):
                    nc.tensor.matmul(
                        pso[:],
                        wo_sb[:, ks, mc * 128 : (mc + 1) * 128],
                        og[:, ks, q2 * 512 : (q2 + 1) * 512],
                        start=(ks == 0),
                        stop=(ks == 3),
                    )
                nc.vector.tensor_scalar_add(
                    out_sb[:, mc, q2 * 512 : (q2 + 1) * 512], pso[:],
                    bgbo_sb[:, 4 + mc : 5 + mc],
                )
            nc.sync.dma_start(out[mc * 128 : (mc + 1) * 128, :], out_sb[:, mc, :])


# ---------------------------------------------------------------------------
# host side
# ---------------------------------------------------------------------------
_NC = None


def _get_nc():
    global _NC
    if _NC is None:
        _NC = build()
    return _NC


def _chunked(w):
    """(512, N) -> (128, 4, N) with row r at [r % 128, r // 128]."""
    n = np.asarray(w).shape[1]
    return np.ascontiguousarray(
        np.asarray(w, np.float32).reshape(4, 128, n).transpose(1, 0, 2)
    ).astype(BF16)


def _ctrans(x):
    """(L, C) -> (128, 4, L) bf16 with channel r at [r % 128, r // 128]."""
    xT = np.asarray(x, np.float32).T  # (C, L)
    return np.ascontiguousarray(
        xT.reshape(4, 128, L).transpose(1, 0, 2)
    ).astype(BF16)


def _prep_inmaps(q_data, k_data, bias, k_mask, Wq, Wk, Wv, Wg, bg, Wo, bo,
                 qcw, qcb, kcw, kcb, vcw, vcb):
    f32 = np.float32
    # exp(bias) -> [hp, h01, p, kc, q] f32: = exp(bias[0, 2hp+h01, q, kc*128+p])
    ebT = np.exp(np.asarray(bias[0], f32)).transpose(0, 2, 1)  # (h, k, q)
    # (h, k, q) -> (hp, h01, kc, p, q) -> (hp, h01, p, kc, q)
    eb_base = np.ascontiguousarray(
        ebT.reshape(4, 2, 8, 128, L).transpose(0, 1, 3, 2, 4)
    )  # (hp, h01, p, kc, q) f32

    wq_a, wk_a, wv_a, wg_a, wo_a = (_chunked(w) for w in (Wq, Wk, Wv, Wg, Wo))
    bgbo = np.zeros((128, 8), f32)
    bgbo[:, 0:4] = np.asarray(bg, f32).reshape(4, 128).T
    bgbo[:, 4:8] = np.asarray(bo, f32).reshape(4, 128).T

    identv = np.eye(128, dtype=f32).astype(BF16)
    selc = np.zeros((2, 128), f32)
    for m in range(128):
        selc[m // 64, m] = 1.0

    # conv taps per core: identity for seqs 0-3, real for 4-7; q scaled D^-0.5
    scale = 1.0 / np.sqrt(D)
    dd = np.arange(128) % 64

    def taps(w3, b1, use_real, s):
        cwc = np.zeros((128, 4), f32)
        if use_real:
            cwc[:, 0:3] = np.asarray(w3, f32)[dd, 0, :] * s
            cwc[:, 3] = np.asarray(b1, f32)[dd] * s
        else:
            cwc[:, 1] = s
        return cwc

    in_maps = []
    for s in range(NCORES):
        real = s >= 4
        cwm = np.concatenate(
            [
                taps(qcw, qcb, real, scale),
                taps(kcw, kcb, real, 1.0),
                taps(vcw, vcb, real, 1.0),
            ],
            axis=1,
        ).astype(f32)
        mk = np.asarray(k_mask[0, s], np.int32).reshape(8, 128).T  # (p, kc)
        ebm = (eb_base * mk.astype(f32)[None, None, :, :, None]).astype(BF16)
        in_maps.append(
            {
                "qd": _ctrans(q_data[0, s]),
                "kd": _ctrans(k_data[0, s]),
                "ebias": ebm.reshape(4, 2, 128, 8 * L),
                "wq": wq_a, "wk": wk_a, "wv": wv_a, "wg": wg_a, "wo": wo_a,
                "convw": cwm,
                "bgbo": bgbo,
                "identv": identv,
                "selc": selc,
            }
        )
    return in_maps


def run(in_maps, trace=False):
    nc = _get_nc()
    return run_bass_kernel_spmd(
        nc, in_maps, core_ids=list(range(NCORES)), trace=trace
    )


def kernel(**inputs):
    in_maps = _prep_inmaps(**inputs)
    res = run(in_maps)
    outp = np.empty((1, NCORES, L, C), np.float32)
    for s in range(NCORES):
        outp[0, s] = np.asarray(res.results[s]["out"], np.float32).T
    return outp
